# revision 4
# baseline (speedup 1.0000x reference)
"""DeepseekV3 mini MoE MLP on 8 TRN2 NeuronCores.

Strategy: expert-parallel. The router (tiny: 0.1% of FLOPs) is computed
with jax ops that mirror the reference bit-for-bit; tokens are then
dispatched on the host to per-expert batches (the "all-to-all"), one
expert per NeuronCore. Each core runs a fused gate/up/silu/mul/down
kernel over its routed tokens. The combine (scatter-add weighted by the
top-k routing weights) happens on the host.

Current design:
- All matmul operands bf16 (PSUM accumulation stays fp32). Max rel err
  vs the fp32 reference ~4e-3 (measured), well inside the 2e-2 gate.
  bf16 halves DMA bytes and SBUF, and enables FWL weight loads (LDW 97ns,
  fully hidden -> matmuls run at the 216ns/512-col roofline).
- x uses a per-tile fully-contiguous DRAM layout, y a d-block-major
  layout, so every DMA is a single dense descriptor. gate+up chunks are
  merged per h-block so the ramp needs only 2 dispatches before compute.
- The remainder tile runs FIRST (fast ramp); ~24 dummy matmuls on zeroed
  tiles run during the initial DMA wait to warm the PE HAM clock gate.
- Token tiles are processed in pairs sharing loaded weights; y for a
  (pair, d-block) goes out as one DMA to keep the sync queue short.
"""

import numpy as np
import ml_dtypes

import concourse.bass as bass
import concourse.mybir as mybir
import concourse.tile as tile
from concourse import bacc
from concourse.bass_utils import run_bass_kernel_spmd

DIM = 1024
HIDDEN = 1024
NUM_EXPERTS = 8
TOP_K = 2
P = 128
TT = 512  # main token tile (PSUM bank = 512 fp32)
DT = DIM // P  # 8 d-tiles
HT = HIDDEN // P  # 8 h-tiles

F32 = mybir.dt.float32
BF16 = mybir.dt.bfloat16
NP_BF16 = ml_dtypes.bfloat16

_program_cache: dict[tuple, object] = {}
LAST_RESULT = None


def _build_expert_program(tiles: tuple):
    """One-expert FFN over groups of token tiles.

    DRAM params (per core), all bf16:
      xt [P, DT*C]    tokens; per tile a contiguous [P, DT, tt] block
                      (p = d % 128, a = d // 128) at offset DT*sum(prev tt)
      wgu [HT, P, 2*DT*P]  per h-blk: gate chunk cols then up chunk cols
      wd [HT, P, HT*P]     down proj chunks: [dout-blk][h-in, h-blk*dout-in]
      yt [P, HT*C]    output, d-block-major: row p, col do*C + tok_index
                      holds y[do*128 + p, tok]
    """
    C = sum(tiles)
    groups = []
    i = 1 if len(tiles) % 2 else 0
    if i:
        groups.append((0,))
    while i < len(tiles):
        groups.append((i, i + 1))
        i += 2

    nc = bacc.Bacc(None, target_bir_lowering=False, debug=False)
    xt = nc.declare_dram_parameter("xt", [P, DT * C], BF16, isOutput=False)
    wgu = nc.declare_dram_parameter("wgu", [HT, P, 2 * DT * P], BF16, isOutput=False)
    wd = nc.declare_dram_parameter("wd", [HT, P, HT * P], BF16, isOutput=False)
    yt = nc.declare_dram_parameter("yt", [P, HT * C], BF16, isOutput=True)

    with tile.TileContext(nc) as tc:
        with (
            tc.tile_pool(name="wpool", bufs=1) as wpool,
            tc.tile_pool(name="xpool", bufs=1) as xpool,
            tc.tile_pool(name="hpool", bufs=1) as hpool,
            tc.tile_pool(name="apool", bufs=3) as apool,
            tc.tile_pool(name="ypool", bufs=4) as ypool,
            tc.tile_pool(name="ppool", bufs=1, space="PSUM") as ppool,
        ):
            # PE warmup: dummy matmuls on zeroed tiles run during the initial
            # DMA wait so the HAM clock gate is at 8/8 when real work lands.
            wz = wpool.tile([P, P], BF16, name="wz", tag="wz")
            xz = wpool.tile([P, 256], BF16, name="xz", tag="xz")
            nc.gpsimd.memset(wz[:, :], 0)
            nc.gpsimd.memset(xz[:, :], 0)
            for i in range(24):
                pz = ppool.tile([P, TT], F32, name="pz", tag=f"py{i % 2}", bufs=2)
                nc.tensor.matmul(pz[:, :256], wz[:, :], xz[:, :], start=True, stop=True)

            # Weight chunk tiles: one per output-column block, so matmuls for
            # block k only depend on chunk k's DMA (fast pipeline ramp).
            wgu_c = [
                wpool.tile([P, 2 * DT * P], BF16, name=f"wgu{k}", tag=f"wgu{k}")
                for k in range(HT)
            ]
            wd_c = [
                wpool.tile([P, HT * P], BF16, name=f"wd{k}", tag=f"wd{k}")
                for k in range(HT)
            ]

            offs = []
            off = 0
            for tt in tiles:
                offs.append(off)
                off += tt

            first = True
            for group in groups:
                tts = [tiles[t] for t in group]
                x_sb = []
                for t, tt in zip(group, tts):
                    xs = xpool.tile(
                        [P, DT * tt], BF16, name=f"x{tt}", tag=f"x{tt}",
                        bufs=(4 if tt == TT else 1),
                    )
                    nc.sync.dma_start(
                        out=xs[:, :],
                        in_=xt.ap()[:, DT * offs[t] : DT * (offs[t] + tt)],
                    )
                    x_sb.append(xs)
                    if first:
                        # first weight chunk right behind the first x tile
                        nc.sync.dma_start(out=wgu_c[0][:, :], in_=wgu.ap()[0])
                        first = False
                        late_w = True

                h_sb = [
                    hpool.tile(
                        [P, HT * tt], BF16, name=f"h{tt}", tag=f"h{i}_{tt}",
                        bufs=(2 if tt == TT else 1),
                    )
                    for i, tt in enumerate(tts)
                ]
                for h in range(HT):
                    pg = [
                        ppool.tile([P, TT], F32, name="pg", tag=f"pg{i}", bufs=1)
                        for i in range(len(group))
                    ]
                    pu = [
                        ppool.tile([P, TT], F32, name="pu", tag=f"pu{i}", bufs=1)
                        for i in range(len(group))
                    ]
                    # interleave the pair so consecutive matmuls share the
                    # stationary weight block
                    for a in range(DT):
                        for i, tt in enumerate(tts):
                            nc.tensor.matmul(
                                pg[i][:, :tt],
                                wgu_c[h][:, a * P : (a + 1) * P],
                                x_sb[i][:, a * tt : (a + 1) * tt],
                                start=(a == 0),
                                stop=(a == DT - 1),
                            )
                    for a in range(DT):
                        for i, tt in enumerate(tts):
                            nc.tensor.matmul(
                                pu[i][:, :tt],
                                wgu_c[h][:, (DT + a) * P : (DT + a + 1) * P],
                                x_sb[i][:, a * tt : (a + 1) * tt],
                                start=(a == 0),
                                stop=(a == DT - 1),
                            )
                    for i, tt in enumerate(tts):
                        act_sb = apool.tile([P, TT], F32, name="act", tag="act")
                        nc.scalar.activation(
                            act_sb[:, :tt], pg[i][:, :tt],
                            mybir.ActivationFunctionType.Silu,
                        )
                        nc.vector.tensor_tensor(
                            h_sb[i][:, h * tt : (h + 1) * tt],
                            act_sb[:, :tt],
                            pu[i][:, :tt],
                            mybir.AluOpType.mult,
                        )
                    if late_w:
                        # issue remaining weight DMAs behind the first h-block's
                        # matmuls, in consumption order
                        for k in range(1, HT):
                            nc.sync.dma_start(out=wgu_c[k][:, :], in_=wgu.ap()[k])
                        for k in range(HT):
                            nc.sync.dma_start(out=wd_c[k][:, :], in_=wd.ap()[k])
                        late_w = False

                gw = sum(tts)
                for do in range(HT):
                    py = [
                        ppool.tile([P, TT], F32, name="py", tag=f"py{i}", bufs=2)
                        for i in range(len(group))
                    ]
                    for a in range(HT):
                        for i, tt in enumerate(tts):
                            nc.tensor.matmul(
                                py[i][:, :tt],
                                wd_c[do][:, a * P : (a + 1) * P],
                                h_sb[i][:, a * tt : (a + 1) * tt],
                                start=(a == 0),
                                stop=(a == HT - 1),
                            )
                    y_sb = ypool.tile([P, 2 * TT], BF16, name="y", tag="y")
                    yo = 0
                    for i, tt in enumerate(tts):
                        nc.scalar.copy(y_sb[:, yo : yo + tt], py[i][:, :tt])
                        yo += tt
                    # tiles in a group are adjacent: one DMA per (group, do)
                    yo0 = do * C + offs[group[0]]
                    nc.sync.dma_start(
                        out=yt.ap()[:, yo0 : yo0 + gw], in_=y_sb[:, :gw]
                    )
    nc.compile()
    return nc


def _tiles_for(max_cnt: int) -> tuple:
    """Token tiles covering max_cnt: one leading remainder tile (multiple of
    64, >=64) then full 512s."""
    full, rem = divmod(max_cnt, TT)
    if rem == 0:
        return (TT,) * max(full, 1)
    rem = ((rem + 63) // 64) * 64
    if rem == TT:
        return (TT,) * (full + 1)
    return (rem,) + (TT,) * full


def _get_program(tiles: tuple):
    if tiles not in _program_cache:
        _program_cache[tiles] = _build_expert_program(tiles)
    return _program_cache[tiles]


def _chunk_w(wt: np.ndarray) -> np.ndarray:
    """[K, M] weight (K contracted) -> chunk layout [m_blk, k_in, k_blk*m_in],
    contiguous per m_blk."""
    K, M = wt.shape
    # wt[k, m] with k = kb*P + kp, m = mb*P + mp  ->  out[mb, kp, kb, mp]
    out = wt.reshape(K // P, P, M // P, P).transpose(2, 1, 0, 3)
    return np.ascontiguousarray(out.reshape(M // P, P, (K // P) * P)).astype(NP_BF16)


def _route(flat: np.ndarray, gate_w: np.ndarray):
    """Mirror the reference router bit-for-bit (jax ops, same backend)."""
    try:
        import jax
        import jax.numpy as jnp

        logits = jnp.asarray(flat) @ jnp.asarray(gate_w).T
        scores = jax.nn.sigmoid(logits)
        top_val, top_idx = jax.lax.top_k(scores, TOP_K)
        top_val = top_val / (top_val.sum(-1, keepdims=True) + 1e-9)
        return np.asarray(top_val), np.asarray(top_idx)
    except Exception:
        # numpy fallback: identical selection semantics (stable descending)
        logits = flat @ gate_w.T
        scores = 1.0 / (1.0 + np.exp(-logits))
        order = np.argsort(-scores, axis=-1, kind="stable")
        top_idx = order[:, :TOP_K].astype(np.int32)
        top_val = np.take_along_axis(scores, top_idx, axis=-1)
        top_val = top_val / (top_val.sum(-1, keepdims=True) + 1e-9)
        return top_val.astype(np.float32), top_idx


def _pack_x(flat_bf: np.ndarray, tok: np.ndarray, tiles: tuple) -> np.ndarray:
    """Gather tokens and lay out as per-tile contiguous [P, DT, tt] blocks."""
    C = sum(tiles)
    xt = np.zeros((P, DT * C), dtype=NP_BF16)
    off = 0
    pos = 0
    for tt in tiles:
        sel = tok[pos : pos + tt]
        n = len(sel)
        if n:
            blk = flat_bf[sel].T.reshape(DT, P, n).transpose(1, 0, 2)  # [P, DT, n]
            dst = xt[:, DT * off : DT * (off + tt)].reshape(P, DT, tt)
            dst[:, :, :n] = blk
        off += tt
        pos += tt
    return xt


def _unpack_y(yt: np.ndarray, C: int, cnt: int) -> np.ndarray:
    """d-block-major yt [P, HT*C] -> [cnt, DIM] fp32."""
    # yt[p, do*C + t] = y[do*P + p, t]
    blk = yt.reshape(P, HT, C)[:, :, :cnt]  # [P, HT, cnt]
    return blk.transpose(1, 0, 2).reshape(DIM, cnt).T.astype(np.float32)


def kernel(x, gate_w, gate_proj, up_proj, down_proj):
    x = np.asarray(x)
    bsz, seqlen, dim = x.shape
    flat = np.ascontiguousarray(x.reshape(-1, dim), dtype=np.float32)
    T = flat.shape[0]
    gate_w = np.asarray(gate_w, dtype=np.float32)
    gate_proj = np.asarray(gate_proj, dtype=np.float32)
    up_proj = np.asarray(up_proj, dtype=np.float32)
    down_proj = np.asarray(down_proj, dtype=np.float32)

    top_val, top_idx = _route(flat, gate_w)

    idx_list = []
    cw_list = []
    for e in range(NUM_EXPERTS):
        mask = top_idx == e  # [T, K]
        tok = np.nonzero(mask.any(axis=1))[0]
        w = (top_val * mask).sum(axis=1)[tok].astype(np.float32)
        idx_list.append(tok)
        cw_list.append(w)

    max_cnt = max(len(i) for i in idx_list)
    tiles = _tiles_for(max_cnt)
    C = sum(tiles)
    nc = _get_program(tiles)

    flat_bf = flat.astype(NP_BF16)
    in_maps = []
    for e in range(NUM_EXPERTS):
        wg_ck = _chunk_w(gate_proj[e].T)  # [HT, P, DT*P]
        wu_ck = _chunk_w(up_proj[e].T)
        in_maps.append(
            {
                "xt": _pack_x(flat_bf, idx_list[e], tiles),
                "wgu": np.ascontiguousarray(
                    np.concatenate([wg_ck, wu_ck], axis=2)
                ),
                "wd": _chunk_w(down_proj[e].T),
            }
        )

    res = run_bass_kernel_spmd(nc, in_maps, core_ids=list(range(NUM_EXPERTS)))
    global LAST_RESULT
    LAST_RESULT = res

    out = np.zeros((T, DIM), dtype=np.float32)
    for e in range(NUM_EXPERTS):
        tok = idx_list[e]
        cnt = len(tok)
        if cnt:
            ye = _unpack_y(res.results[e]["yt"], C, cnt)
            out[tok] += ye * cw_list[e][:, None]
    return out.reshape(bsz, seqlen, dim)
